# revision 25
# baseline (speedup 1.0000x reference)
"""Trainium2 Bass kernel for DecoderCrossAttention (B=8, S=2048, T=1024, E=1024, C=768, H=16, D=64).

Data-parallel over batch: 8 NeuronCores, one batch element each, no collectives.

v7: restructured from v6 around three wins found via TimelineSim cost analysis:
  - sums matmuls eliminated: v0 carries a 65th column of 16.0 per head, so the
    per-head attn@v accumulation also produces 16*rowsum(e) in psum row 64
    (matmul cost depends only on out free size, so the extra column is free).
    Normalization is deferred: attn@v consumes RAW e; po and e are scaled by
    r/16 afterwards.
  - avg_attn and out are written TRANSPOSED to DRAM ([T,S] bf16 / [E,S] f32)
    and untransposed on host -> no tail PE transposes / ACT copies at all;
    acc ([t,s] layout) DMAs straight out.
  - the avg accumulation over pairs runs on the DMA engines (gpsimd SWDGE
    cce add, <=4KB/partition per DMA), freeing ~130us of DVE time; only the
    per-pair normalize multiply stays on DVE (2x_1p tensor_tensor).
  - oproj bias is folded into the psum->sbuf copies via a gathered per-
    partition bo_eff column (no bias matmuls); r' broadcasts cast f32->bf16
    in the DMA itself.

Carried from v6: ones-column = 16.0 folds the 1/H of avg_attn; Wo scaled by
16; bo_eff = bv@Wo + bo; per-pair recip/broadcast pipelining; chunk tails
software-pipelined into the next chunk; xT/qproj emitted one chunk ahead;
phase-A DMAs spread across queues.
"""

import sys

sys.path.insert(0, "/opt/trn_rl_repo")

from contextlib import ExitStack

import numpy as np

import concourse.bass as bass
import concourse.mybir as mybir
import concourse.tile as tile
from concourse import bacc
from concourse.bass_utils import run_bass_kernel_spmd
from concourse.masks import make_identity

F32 = mybir.dt.float32
BF16 = mybir.dt.bfloat16
AF = mybir.ActivationFunctionType
OP = mybir.AluOpType

N_CORES = 8
S, T, E, C = 2048, 1024, 1024, 768
H, D = 16, 64
DP = D + 1  # v0 column pitch per head (64 d + 1 ones col)
P = 128
SC = 256  # S-chunk size
NCH = S // SC
KE = E // P  # 8
KC = C // P  # 6
TT = T // P  # 8
SCALE = 0.125

_PROGRAM = None


def _bcast_dma(nc, eng, out_t, src_row):
    """Broadcast a [1, ...] SBUF row to [rows, ...] via DMA (zero-step dim)."""
    src_b = bass.AP(
        tensor=src_row.tensor,
        offset=src_row.offset,
        ap=[list(src_row.ap[0]), [0, out_t.shape[0]]] + [list(d) for d in src_row.ap[1:]],
    )
    eng.dma_start(out=out_t, in_=src_b)


def build_program(loop_iters=0, nch=None):
    nc = bacc.Bacc("TRN2", target_bir_lowering=False, debug=False, num_devices=N_CORES)

    x = nc.dram_tensor("x", [S, E], F32, kind="ExternalInput").ap()
    enc = nc.dram_tensor("enc", [T, C], F32, kind="ExternalInput").ap()
    Wq = nc.dram_tensor("Wq", [E, E], F32, kind="ExternalInput").ap()
    bq = nc.dram_tensor("bq", [E], F32, kind="ExternalInput").ap()
    Wk = nc.dram_tensor("Wk", [C, E], F32, kind="ExternalInput").ap()
    bk = nc.dram_tensor("bk", [E], F32, kind="ExternalInput").ap()
    Wv = nc.dram_tensor("Wv", [C, E], F32, kind="ExternalInput").ap()
    bv = nc.dram_tensor("bv", [E], F32, kind="ExternalInput").ap()
    Wo = nc.dram_tensor("Wo", [E, E], F32, kind="ExternalInput").ap()
    bo = nc.dram_tensor("bo", [E], F32, kind="ExternalInput").ap()
    out_t = nc.dram_tensor("out_t", [E, S], F32, kind="ExternalOutput").ap()
    avg_t = nc.dram_tensor("avg_t", [T, S], BF16, kind="ExternalOutput").ap()

    global NCH
    old_nch = NCH
    if nch is not None:
        NCH = nch
    try:
        with tile.TileContext(nc) as tc:
            if loop_iters:
                with tc.For_i(0, loop_iters, 1):
                    _build(tc, x, enc, Wq, bq, Wk, bk, Wv, bv, Wo, bo, out_t, avg_t)
            else:
                _build(tc, x, enc, Wq, bq, Wk, bk, Wv, bv, Wo, bo, out_t, avg_t)
    finally:
        NCH = old_nch
    nc.compile()
    return nc


def _build(tc, x, enc, Wq, bq, Wk, bk, Wv, bv, Wo, bo, out_t, avg_t):
    nc = tc.nc
    out_tr = out_t.rearrange("(a p) s -> p a s", p=P)  # [P, KE, S]
    avg_tr = avg_t.rearrange("(a p) s -> p a s", p=P)  # [P, TT, S]
    with ExitStack() as stack:
        consts = stack.enter_context(tc.tile_pool(name="consts", bufs=1))
        resident = stack.enter_context(tc.tile_pool(name="resident", bufs=1))

        ident_b = consts.tile([P, P], BF16)
        make_identity(nc, ident_b)
        bq_sb = consts.tile([P, KE], F32)
        nc.sync.dma_start(out=bq_sb, in_=bq.rearrange("(m p) -> p m", p=P))
        bk_sb = consts.tile([P, KE], F32)
        nc.sync.dma_start(out=bk_sb, in_=bk.rearrange("(m p) -> p m", p=P))
        bv_sb = consts.tile([P, KE], F32)
        nc.sync.dma_start(out=bv_sb, in_=bv.rearrange("(m p) -> p m", p=P))
        bo_row = consts.tile([1, E], F32)
        nc.sync.dma_start(out=bo_row, in_=bo[None, :])
        bo_eff_f32 = consts.tile([1, E], F32)
        bo_eff_col = consts.tile([P, KE], F32)

        Wq_bf = resident.tile([P, KE, E], BF16)
        Wo_bf = resident.tile([P, KE, E], BF16)  # holds 16*Wo
        kT_bf = resident.tile([P, KE, T], BF16)
        v0_sb = resident.tile([P, TT, H * DP], BF16)  # per head: 64 d + ones col

        # pools that must exist during phase A (chunk-0 head + pair-0 scores)
        xload = stack.enter_context(tc.tile_pool(name="xload", bufs=2))
        xt_pool = stack.enter_context(tc.tile_pool(name="xt_pool", bufs=2))
        qt_pool = stack.enter_context(tc.tile_pool(name="qt_pool", bufs=2))
        et_pool = stack.enter_context(tc.tile_pool(name="et_pool", bufs=4))
        ps_ring = stack.enter_context(tc.tile_pool(name="ps_ring", bufs=2, space="PSUM"))
        ps_sc = stack.enter_context(tc.tile_pool(name="ps_sc", bufs=2, space="PSUM"))

        x_tiles = {}
        qt_tiles = {}

        def emit_xdma(c):
            s0 = c * SC
            x_sb = xload.tile([P, 2, E], F32, tag="x_sb", name="x_sb")
            nc.sync.dma_start(
                out=x_sb, in_=x[s0 : s0 + SC, :].rearrange("(a p) e -> p a e", p=P)
            )
            x_tiles[c] = x_sb

        xt_tiles = {}

        def emit_xt(c):
            x_sb = x_tiles.pop(c)
            x_bf = xt_pool.tile([P, 2, E], BF16, tag="x_bf", name="x_bf", bufs=1)
            nc.scalar.copy(x_bf, x_sb)
            xT_c = xt_pool.tile([P, KE, SC], BF16)
            for e8 in range(KE):
                ps_m = ps_ring.tile([P, SC], BF16, tag="big", name="ps_xt")
                for s2 in range(2):
                    nc.tensor.transpose(
                        ps_m[:, s2 * P : (s2 + 1) * P],
                        x_bf[:, s2, e8 * P : (e8 + 1) * P],
                        ident_b,
                    )
                nc.vector.tensor_copy(xT_c[:, e8, :], ps_m)
            xt_tiles[c] = xT_c

        def emit_qproj(c):
            xT_c = xt_tiles.pop(c)
            qT_c = qt_pool.tile([P, KE, SC], BF16)
            for m8 in range(KE):
                ps_m = ps_ring.tile([P, SC], F32, tag="big", name="ps_q")
                for k8 in range(KE):
                    nc.tensor.matmul(
                        ps_m,
                        Wq_bf[:, k8, m8 * P : (m8 + 1) * P],
                        xT_c[:, k8, :],
                        start=(k8 == 0),
                        stop=(k8 == KE - 1),
                    )
                nc.vector.tensor_scalar_add(qT_c[:, m8, :], ps_m, bq_sb[:, m8 : m8 + 1])
            qt_tiles[c] = qT_c

        def emit_pair_scores(pr, qT_c):
            """Scores + exp for pair pr (two heads, tile_position packed)."""
            eT_pair = et_pool.tile([P, 2, TT, SC], BF16, tag="eT", name="eT_pair")
            for half in range(2):
                ps_h = {
                    0: ps_sc.tile([P, 4, SC], F32, tag="sc", name="ps_sc_a"),
                    1: ps_sc.tile([P, 4, SC], F32, tag="sc", name="ps_sc_b"),
                }
                for t4 in range(4):
                    t8 = half * 4 + t4
                    for hh, tp in ((0, 0), (1, 64)):
                        nc.tensor.matmul(
                            ps_h[hh][:, t4, :],
                            kT_bf[tp : tp + D, pr, t8 * P : (t8 + 1) * P],
                            qT_c[tp : tp + D, pr, :],
                            start=True,
                            stop=True,
                            tile_position=(tp, 0),
                        )
                for hh in range(2):
                    nc.scalar.activation(
                        eT_pair[:, hh, half * 4 : half * 4 + 4, :],
                        ps_h[hh],
                        AF.Exp,
                        scale=SCALE,
                    )
            return eT_pair

        # ---------------- phase A ----------------
        with ExitStack() as ph:
            early = ph.enter_context(tc.tile_pool(name="early", bufs=1))
            ld_pool = ph.enter_context(tc.tile_pool(name="ld_pool", bufs=2))

            Wk_bf = early.tile([P, KC, E], BF16)
            Wv_bf = early.tile([P, KC, E], BF16)
            encT_bf = early.tile([P, KC, T], BF16)

            # enc + Wk first: kT is the critical path for pair-0 scores.
            # enc -> bf16 -> PE-transpose -> encT
            for t8 in range(TT):
                enc_t = ld_pool.tile([P, C], F32, tag="lde", name="enc_t")
                nc.scalar.dma_start(out=enc_t, in_=enc[t8 * P : (t8 + 1) * P, :])
                enc_b = ld_pool.tile([P, C], BF16, tag="ldeb", name="enc_b")
                nc.scalar.copy(enc_b, enc_t)
                ps_a = ps_ring.tile([P, C], BF16, tag="big", name="ps_tr")
                for c6 in range(KC):
                    nc.tensor.transpose(
                        ps_a[:, c6 * P : (c6 + 1) * P],
                        enc_b[:, c6 * P : (c6 + 1) * P],
                        ident_b,
                    )
                nc.vector.tensor_copy(
                    encT_bf[:, :, t8 * P : (t8 + 1) * P],
                    ps_a.rearrange("p (k t) -> p k t", k=KC),
                )
            for k6 in range(KC):
                wk_t = ld_pool.tile([P, E], F32, tag="ldk", name="wk_t")
                nc.scalar.dma_start(out=wk_t, in_=Wk[k6 * P : (k6 + 1) * P, :])
                nc.vector.tensor_copy(Wk_bf[:, k6, :], wk_t)
            emit_xdma(0)  # prefetch chunk-0 x on the idle sync queue
            for k8 in range(KE):
                wq_t = ld_pool.tile([P, E], F32, tag="ld", name="wq_t")
                nc.scalar.dma_start(out=wq_t, in_=Wq[k8 * P : (k8 + 1) * P, :])
                nc.scalar.copy(Wq_bf[:, k8, :], wq_t)
            for k6 in range(KC):
                wv_t = ld_pool.tile([P, E], F32, tag="ldk", name="wv_t")
                nc.scalar.dma_start(out=wv_t, in_=Wv[k6 * P : (k6 + 1) * P, :])
                nc.vector.tensor_copy(Wv_bf[:, k6, :], wv_t)

            # kT[e',t] = sum_c Wk[c,e'] encT[c,t], +bk, store bf16 (half tiles)
            for m8 in range(KE):
                for n2 in range(2):
                    ps_m = ps_ring.tile([P, 512], F32, tag="big", name="ps_k")
                    for c6 in range(KC):
                        nc.tensor.matmul(
                            ps_m,
                            Wk_bf[:, c6, m8 * P : (m8 + 1) * P],
                            encT_bf[:, c6, n2 * 512 : (n2 + 1) * 512],
                            start=(c6 == 0),
                            stop=(c6 == KC - 1),
                        )
                    nc.scalar.activation(
                        kT_bf[:, m8, n2 * 512 : (n2 + 1) * 512],
                        ps_m,
                        AF.Identity,
                        bias=bk_sb[:, m8 : m8 + 1],
                    )

            # chunk-0 head + pair-0 scores/exp start the softmax pipeline early
            emit_xt(0)
            emit_qproj(0)
            eT_p0 = emit_pair_scores(0, qt_tiles[0])

            # ones columns of v0 (16x: folds 1/H and makes r' = 1/(16*rowsum))
            nc.vector.memset(
                v0_sb.rearrange("p a (h d) -> p a h d", d=DP)[:, :, :, D : D + 1], 16.0
            )

            # v0[t,e'] = sum_c encT[c,t] Wv[c,e'] (no bias: folded into bo_eff);
            # psum [t, 512-e' chunk] lands in v0 as 8 strided 64-wide head cols
            v0r = v0_sb.rearrange("p a (h d) -> p a h d", d=DP)
            for t8 in range(TT):
                for n2 in range(2):
                    ps_m = ps_ring.tile([P, 512], F32, tag="big", name="ps_v")
                    for c6 in range(KC):
                        nc.tensor.matmul(
                            ps_m,
                            encT_bf[:, c6, t8 * P : (t8 + 1) * P],
                            Wv_bf[:, c6, n2 * 512 : (n2 + 1) * 512],
                            start=(c6 == 0),
                            stop=(c6 == KC - 1),
                        )
                    dst = v0r[:, t8, n2 * 8 : (n2 + 1) * 8, 0:D]
                    if t8 % 2 == 0:
                        nc.scalar.copy(dst, ps_m.rearrange("p (h d) -> p h d", d=D))
                    else:
                        nc.vector.tensor_copy(
                            dst, ps_m.rearrange("p (h d) -> p h d", d=D)
                        )

            # Wo (x16) and bo_eff = bv @ Wo + bo
            for k8 in range(KE):
                wo_t = ld_pool.tile([P, E], F32, tag="ld", name="wo_t")
                nc.scalar.dma_start(out=wo_t, in_=Wo[k8 * P : (k8 + 1) * P, :])
                nc.vector.tensor_scalar_mul(Wo_bf[:, k8, :], wo_t, 16.0)
            bv_bf = early.tile([P, KE], BF16)
            nc.vector.tensor_copy(bv_bf, bv_sb)
            for n2 in range(2):
                ps_m = ps_ring.tile([1, 512], F32, tag="big", name="ps_b")
                for k8 in range(KE):
                    nc.tensor.matmul(
                        ps_m,
                        bv_bf[:, k8 : k8 + 1],
                        Wo_bf[:, k8, n2 * 512 : (n2 + 1) * 512],
                        start=(k8 == 0),
                        stop=(k8 == KE - 1),
                    )
                nc.vector.scalar_tensor_tensor(
                    bo_eff_f32[:, n2 * 512 : (n2 + 1) * 512],
                    ps_m,
                    1.0 / 16.0,
                    bo_row[:, n2 * 512 : (n2 + 1) * 512],
                    OP.mult,
                    OP.add,
                )
            # gather bo_eff into per-partition column form [p, k8] so the
            # oproj bias rides the psum->sbuf copies instead of a matmul
            for k8 in range(KE):
                seg = bo_eff_f32[:, k8 * P : (k8 + 1) * P]
                gather_src = bass.AP(
                    tensor=seg.tensor,
                    offset=seg.offset,
                    ap=[list(seg.ap[0]), [1, P], [1, 1]],
                )
                nc.sync.dma_start(out=bo_eff_col[:, k8 : k8 + 1], in_=gather_src)

        # ---------------- phase B: S-chunk loop (tail software-pipelined) ----
        with ExitStack() as mn:
            wb_pool = mn.enter_context(tc.tile_pool(name="wb_pool", bufs=6))
            st_pool = mn.enter_context(tc.tile_pool(name="st_pool", bufs=2))
            acc2_pool = mn.enter_context(tc.tile_pool(name="acc2_pool", bufs=1))
            acc_pool = mn.enter_context(tc.tile_pool(name="acc_pool", bufs=2))
            ot_pool = mn.enter_context(tc.tile_pool(name="ot_pool", bufs=2))
            out_pool = mn.enter_context(tc.tile_pool(name="out_pool", bufs=2))
            ps_po = mn.enter_context(tc.tile_pool(name="ps_po", bufs=2, space="PSUM"))

            def emit_tail(s0, outT, acc):
                # final projection of the previous chunk: out^T[e, s] chunks
                out_sb = out_pool.tile([P, KE, SC], F32, tag="out_sb", name="out_sb")
                for eb in range(KE):
                    ps_m = ps_ring.tile([P, SC], F32, tag="big", name="ps_o")
                    for k8 in range(KE):
                        nc.tensor.matmul(
                            ps_m,
                            Wo_bf[:, k8, eb * P : (eb + 1) * P],
                            outT[:, k8, :],
                            start=(k8 == 0),
                            stop=(k8 == KE - 1),
                        )
                    if eb % 2 == 0:
                        nc.scalar.activation(
                            out_sb[:, eb, :], ps_m, AF.Identity,
                            bias=bo_eff_col[:, eb : eb + 1],
                        )
                    else:
                        nc.vector.tensor_scalar_add(
                            out_sb[:, eb, :], ps_m, bo_eff_col[:, eb : eb + 1]
                        )
                nc.sync.dma_start(out=out_tr[:, :, s0 : s0 + SC], in_=out_sb)
                # avg^T: acc is already [t, s] with 1/(16*rowsum) applied
                nc.scalar.dma_start(out=avg_tr[:, :, s0 : s0 + SC], in_=acc)

            tail = None
            for c in range(NCH):
                s0 = c * SC
                if c + 1 < NCH:
                    emit_xdma(c + 1)
                qT_c = qt_tiles.pop(c)

                acc2 = acc2_pool.tile([P, 2, TT, SC], BF16)
                outT = ot_pool.tile([P, KE, SC], BF16)
                n_dve = 0

                for q in range(H // 4):  # quads of 4 heads
                    for pp in range(2):
                        pr = 2 * q + pp
                        if c == 0 and pr == 0:
                            eT_pair = eT_p0
                        else:
                            eT_pair = emit_pair_scores(pr, qT_c)

                        if c > 0 and q == 0 and pp == 0:
                            # previous chunk's projection + avg DMA fill the
                            # bubble while this chunk's softmax chain warms up
                            emit_tail(s0 - SC, tail[0], tail[1])

                        # attn@v on RAW e; v0's 16.0 column makes psum row 64
                        # = 16*rowsum -> r' = 1/(16*rowsum) for free. Both
                        # heads share ONE full psum bank so no other pair's
                        # PE writes can collide with our DVE reads of it.
                        po = ps_po.tile([DP, 2, SC], F32, tag="po", name="po")
                        for hh in range(2):
                            h = 2 * pr + hh
                            for t8 in range(TT):
                                nc.tensor.matmul(
                                    po[:, hh, :],
                                    v0_sb[:, t8, h * DP : (h + 1) * DP],
                                    eT_pair[:, hh, t8, :],
                                    start=(t8 == 0),
                                    stop=(t8 == TT - 1),
                                )

                        # plain po->outT copies first: frees the po psum bank
                        # without waiting on the r' broadcast round-trip
                        for hh in range(2):
                            nc.vector.tensor_copy(
                                outT[64 * hh : 64 * hh + 64, pr, :],
                                po[0:D, hh, :],
                            )
                        # custom-DVE ops mishandle nonzero base partitions:
                        # stage the sums row to a base-0 SBUF tile first
                        sums_sb = st_pool.tile([1, 2, SC], F32, tag="sums")
                        nc.vector.tensor_copy(sums_sb, po[D : D + 1, :, :])
                        rq = st_pool.tile([1, 2, SC], F32, tag="rq")
                        nc.vector.reciprocal_approx_fast(out=rq, in_=sums_sb)
                        # broadcast r' to all 128 partitions; the gpsimd DMA
                        # casts f32 -> bf16 in flight
                        wb = wb_pool.tile([P, 2, SC], BF16, tag="wb", name="wb_t")
                        _bcast_dma(nc, nc.gpsimd, wb, rq)

                        # outT[e',s] *= r' (16*Wo compensates the /16). wb
                        # rows are all identical, so read the slice at the
                        # same base partition as outT (verifier requirement).
                        for hh in range(2):
                            nc.vector.tensor_tensor(
                                outT[64 * hh : 64 * hh + 64, pr, :],
                                outT[64 * hh : 64 * hh + 64, pr, :],
                                wb[64 * hh : 64 * hh + 64, hh, :],
                                OP.mult,
                            )

                        # e *= r' in place (for avg only; 2x_1p TT), one
                        # head at a time so the in-order DVE queue can slot
                        # the next pair's po copies between the halves
                        a_pair = eT_pair
                        for hh in range(2):
                            wb_h = wb[:, hh, None, :].to_broadcast([P, TT, SC])
                            nc.vector.tensor_tensor(
                                a_pair[:, hh], eT_pair[:, hh], wb_h, OP.mult
                            )

                        # avg accumulation: first pair lands via a 4x DVE copy,
                        # the rest accumulate on the DMA engines (SWDGE cce
                        # add). One accumulate DMA must stay <= 4KB/partition
                        # (8KB hits NRT_EXEC_UNIT_UNRECOVERABLE), so split hh.
                        import os

                        if n_dve == 0:
                            nc.vector.tensor_copy(acc2, a_pair)
                            n_dve = 1
                        elif os.environ.get("V7_NO_ACCUM"):
                            nc.vector.tensor_tensor(acc2, acc2, a_pair, OP.add)
                            n_dve += 1
                        else:
                            for hh in range(2):
                                nc.gpsimd.dma_start(
                                    out=acc2[:, hh], in_=a_pair[:, hh],
                                    accum_op=OP.add,
                                )
                            n_dve += 1

                        if q == 1 and pp == 1 and c + 1 < NCH:
                            # next chunk's transposes and q-projection are
                            # injected at two separate points to smooth the
                            # PE/DVE interleave across the chunk
                            emit_xt(c + 1)
                        if q == H // 4 - 2 and pp == 1 and c + 1 < NCH:
                            emit_qproj(c + 1)

                # merge + fold pair dim: acc[t8, s] = acc2[:,0] + acc2[:,1]
                acc = acc_pool.tile([P, TT, SC], BF16)
                nc.vector.tensor_tensor(acc, acc2[:, 0], acc2[:, 1], OP.add)
                tail = (outT, acc)

            emit_tail((NCH - 1) * SC, tail[0], tail[1])


def get_program():
    global _PROGRAM
    if _PROGRAM is None:
        _PROGRAM = build_program()
    return _PROGRAM


def kernel(**inputs):
    nc = get_program()
    common = {
        k: np.ascontiguousarray(np.asarray(inputs[k], dtype=np.float32))
        for k in ("Wq", "bq", "Wk", "bk", "Wv", "bv", "Wo", "bo")
    }
    x = np.asarray(inputs["x"], dtype=np.float32)
    enc = np.asarray(inputs["encoder_output"], dtype=np.float32)
    in_maps = [
        dict(common, x=np.ascontiguousarray(x[b]), enc=np.ascontiguousarray(enc[b]))
        for b in range(N_CORES)
    ]
    res = run_bass_kernel_spmd(nc, in_maps, list(range(N_CORES)))
    out = np.stack(
        [np.asarray(res.results[b]["out_t"]).T for b in range(N_CORES)]
    )
    avg = np.stack(
        [np.asarray(res.results[b]["avg_t"], dtype=np.float32).T for b in range(N_CORES)]
    )
    return out, avg
